# revision 3
# baseline (speedup 1.0000x reference)
"""EXL3 trellis-quantized linear layer on 8 Trainium2 NeuronCores.

y = Had(Had(x*suh) @ dequant(trellis)) * svh + bias

Sharding: column-parallel over output features (N). Each of the 8 cores
dequants and multiplies its 1792-column shard; host concatenates.

Decode pipeline per weight (t = column-within-tile class, fixed shift r):
    state = ((A & M1) << r) | (B >> (16-r))     A,B = trellis word pair
    z     = (state*89226354 + 64248484) mod 2^32   (gpsimd int32 TT ops)
    z    &= 0x8FFF8FFF
    w     = fp16(z_lo) + fp16(z_hi)
The fp16 halves are never summed explicitly: the masked z tile is bitcast
to fp16 and streamed into the PE as two rhs streams (lo/hi interleaved)
accumulating into the same PSUM bank.

Weight (j,t) of tile (Tk,Tn) sits at W[16Tk+j, 16Tn+t], so an output
column's weights share one t class. PSUM columns are produced t-major and
the output Hadamard uses a row-permuted H to compensate.
"""

import sys

if "/opt/trn_rl_repo" not in sys.path:
    sys.path.insert(0, "/opt/trn_rl_repo")

import numpy as np

import concourse.bacc as bacc
import concourse.mybir as mybir
from concourse import tile
from concourse.bass_utils import run_bass_kernel_spmd

AL = mybir.AluOpType
DT = mybir.dt

# problem geometry (hardcoded per contest contract)
K = 4096
N = 14336
BATCH = 8
NCORES = 8
TNC = (N // 16) // NCORES  # 112 trellis tile-cols per core
NC_COLS = TNC * 16  # 1792 out features per core
NSLAB = 7  # 16 tile-cols per slab
KC = 32  # 128-row k-chunks

LCG_Q = 89226354
LCG_D = 64248484
MASK32 = np.int32(np.uint32(0x8FFF8FFF).astype(np.int64) - (1 << 32))

# per-class constants
CLS = []
for t in range(16):
    c = (3 * t) // 16
    r = 3 * t - 16 * c
    CLS.append((c, r))


def _hadamard128():
    h = np.array([[1.0]], dtype=np.float64)
    while h.shape[0] < 128:
        h = np.block([[h, h], [h, -h]])
    return (h / np.sqrt(128.0)).astype(np.float32)


def _perm_h():
    # psum col f' = t*8 + sub  <->  true in-block col sub*16 + t
    h = _hadamard128()
    pi = np.zeros(128, dtype=np.int64)
    for t in range(16):
        for sub in range(8):
            pi[t * 8 + sub] = sub * 16 + t
    return np.ascontiguousarray(h[pi, :])


_NC_CACHE = None


def _build_program():
    global _NC_CACHE
    if _NC_CACHE is not None:
        return _NC_CACHE

    nc = bacc.Bacc("TRN2", target_bir_lowering=False, debug=False)

    d_planes = nc.dram_tensor("planes", [128, 4 * NSLAB * 512], DT.uint16, kind="ExternalInput")
    d_xT = nc.dram_tensor("xT", [128, KC * BATCH], DT.float16, kind="ExternalInput")
    d_suhT = nc.dram_tensor("suhT", [128, KC], DT.float16, kind="ExternalInput")
    d_H = nc.dram_tensor("Hmat", [128, 128], DT.float32, kind="ExternalInput")
    d_HP = nc.dram_tensor("HP", [128, 128], DT.float32, kind="ExternalInput")
    d_ident = nc.dram_tensor("ident8", [8, 8], DT.float32, kind="ExternalInput")
    d_svh = nc.dram_tensor("svhb", [8, NC_COLS], DT.float32, kind="ExternalInput")
    d_bias = nc.dram_tensor("biasb", [8, NC_COLS], DT.float32, kind="ExternalInput")
    d_out = nc.dram_tensor("out", [8, NC_COLS], DT.float16, kind="ExternalOutput")

    with tile.TileContext(nc) as tc:
        with (
            tc.tile_pool(name="const", bufs=1) as cpool,
            tc.tile_pool(name="planes", bufs=1) as ppool,
            tc.tile_pool(name="cls", bufs=3) as clspool,
            tc.tile_pool(name="lcg", bufs=3) as lcgpool,
            tc.tile_pool(name="zslab", bufs=2) as zpool,
            tc.tile_pool(name="outp", bufs=1) as opool,
            tc.tile_pool(name="psum", bufs=2, space="PSUM") as pspool,
            tc.tile_pool(name="psum_s", bufs=2, space="PSUM") as pspool_s,
        ):
            # ---- constants / small inputs ----
            planes = ppool.tile([128, 4 * NSLAB * 512], DT.uint16, tag="planes")
            for c4 in range(4):
                sl = slice(c4 * NSLAB * 512, (c4 + 1) * NSLAB * 512)
                nc.sync.dma_start(planes[:, sl], d_planes[:, sl])
            t_xT = cpool.tile([128, KC * BATCH], DT.float16, tag="xT")
            t_suhT = cpool.tile([128, KC], DT.float16, tag="suhT")
            t_H = cpool.tile([128, 128], DT.float32, tag="H")
            t_HP = cpool.tile([128, 128], DT.float32, tag="HP")
            t_id8 = cpool.tile([8, 8], DT.float32, tag="id8")
            t_svh = cpool.tile([8, NC_COLS], DT.float32, tag="svh")
            t_bias = cpool.tile([8, NC_COLS], DT.float32, tag="bias")
            nc.sync.dma_start(t_xT[:], d_xT[:])
            nc.sync.dma_start(t_suhT[:], d_suhT[:])
            nc.sync.dma_start(t_H[:], d_H[:])
            nc.sync.dma_start(t_HP[:], d_HP[:])
            nc.sync.dma_start(t_id8[:], d_ident[:])
            nc.sync.dma_start(t_svh[:], d_svh[:])
            nc.sync.dma_start(t_bias[:], d_bias[:])

            t_q = cpool.tile([128, 1], DT.int32, tag="cq")
            t_d = cpool.tile([128, 1], DT.int32, tag="cd")
            t_m = cpool.tile([128, 1], DT.int32, tag="cm")
            nc.vector.memset(t_q[:], LCG_Q)
            nc.vector.memset(t_d[:], LCG_D)
            nc.vector.memset(t_m[:], int(MASK32))

            # ---- input rotation: xhT[j, kc*8+b] ----
            t_xsT = cpool.tile([128, KC * BATCH], DT.float32, tag="xsT")
            nc.vector.tensor_tensor(
                t_xsT[:].rearrange("p (kc b) -> p kc b", kc=KC),
                t_xT[:].rearrange("p (kc b) -> p kc b", kc=KC),
                t_suhT[:].unsqueeze(2).broadcast_to([128, KC, BATCH]),
                AL.mult,
            )
            ps_xh = pspool.tile([128, KC * BATCH], DT.float32, tag="ps_xh")
            nc.tensor.matmul(ps_xh[:], t_H[:], t_xsT[:], start=True, stop=True)
            t_xhT = cpool.tile([128, KC * BATCH], DT.float16, tag="xhT")
            nc.scalar.copy(t_xhT[:], ps_xh[:])

            t_out = opool.tile([8, NC_COLS], DT.float16, tag="outsb")

            # ---- main loop over 7 slabs ----
            for slab in range(NSLAB):
                tz = zpool.tile([128, 16 * 512], DT.int32, tag="z")
                for t16, (c, r) in enumerate(CLS):
                    a_v = planes[:, c * NSLAB * 512 + slab * 512 : c * NSLAB * 512 + (slab + 1) * 512]
                    b_v = planes[:, (c + 1) * NSLAB * 512 + slab * 512 : (c + 1) * NSLAB * 512 + (slab + 1) * 512]
                    m1 = (1 << (16 - r)) - 1
                    t_x1 = clspool.tile([128, 512], DT.uint16, tag="x1")
                    t_x2 = clspool.tile([128, 512], DT.uint16, tag="x2")
                    t_st = clspool.tile([128, 512], DT.int32, tag="st")
                    # X1 = (A & M1) << r ; X2 = B >> (16-r)
                    nc.vector.tensor_scalar(
                        t_x1[:], a_v, m1, r, AL.bitwise_and, AL.logical_shift_left
                    )
                    nc.vector.tensor_scalar(
                        t_x2[:], b_v, 16 - r, None, AL.logical_shift_right
                    )
                    # state (disjoint bits: add == or), widen to i32 via fp32
                    nc.vector.tensor_tensor(t_st[:], t_x1[:], t_x2[:], AL.add)
                    # LCG on gpsimd (exact int32 wraparound)
                    t_g1 = lcgpool.tile([128, 512], DT.int32, tag="g1")
                    t_g2 = lcgpool.tile([128, 512], DT.int32, tag="g2")
                    nc.gpsimd.tensor_tensor(
                        t_g1[:], t_st[:], t_q[:].broadcast_to([128, 512]), AL.mult
                    )
                    nc.gpsimd.tensor_tensor(
                        t_g2[:], t_g1[:], t_d[:].broadcast_to([128, 512]), AL.add
                    )
                    nc.vector.tensor_scalar(
                        tz[:, t16 * 512 : (t16 + 1) * 512],
                        t_g2[:],
                        int(MASK32),
                        None,
                        AL.bitwise_and,
                    )

                # ---- matmuls: 2 fp16 streams x 32 k-chunks ----
                ps_y = pspool.tile([8, 256], DT.float32, tag="ps_y")
                zf = tz[:].bitcast(DT.float16).rearrange(
                    "p (t kc b sub x) -> p kc x b t sub", t=16, kc=KC, b=2, sub=8, x=2
                )
                n_mm = 2 * KC
                i_mm = 0
                for xi in range(2):
                    for kc in range(KC):
                        nc.tensor.matmul(
                            ps_y[:],
                            t_xhT[:, kc * BATCH : (kc + 1) * BATCH],
                            zf[:, kc, xi],
                            start=(i_mm == 0),
                            stop=(i_mm == n_mm - 1),
                        )
                        i_mm += 1

                # ---- tail: transpose + permuted Hadamard + svh/bias ----
                t_y = clspool.tile([8, 256], DT.float32, tag="ysb")
                nc.scalar.copy(t_y[:], ps_y[:])
                for bb in range(2):
                    nblk = slab * 2 + bb
                    ps_t = pspool_s.tile([128, 8], DT.float32, tag="ps_t")
                    nc.tensor.transpose(
                        ps_t[:], t_y[:, bb * 128 : (bb + 1) * 128], t_id8[:]
                    )
                    t_yT = clspool.tile([128, 8], DT.float32, tag="yT")
                    nc.vector.tensor_copy(t_yT[:], ps_t[:])
                    ps_h = pspool_s.tile([8, 128], DT.float32, tag="ps_h")
                    nc.tensor.matmul(ps_h[:], t_yT[:], t_HP[:], start=True, stop=True)
                    t_f = clspool.tile([8, 128], DT.float32, tag="fin")
                    nc.vector.tensor_tensor(
                        t_f[:], ps_h[:], t_svh[:, nblk * 128 : (nblk + 1) * 128], AL.mult
                    )
                    nc.vector.tensor_tensor(
                        t_out[:, nblk * 128 : (nblk + 1) * 128],
                        t_f[:],
                        t_bias[:, nblk * 128 : (nblk + 1) * 128],
                        AL.add,
                    )

            nc.sync.dma_start(d_out[:], t_out[:])

    nc.compile()
    _NC_CACHE = nc
    return nc


def _prep_core_inputs(x, trellis, suh, svh, bias, core):
    tshard = trellis[:, core * TNC : (core + 1) * TNC, :]  # [256, 112, 48]
    j = np.arange(16)
    planes = np.empty((128, 4 * NSLAB * 512), dtype=np.uint16)
    for c in range(4):
        w = (3 * j + c) % 48
        pl = tshard[:, :, w]  # [256 Tk, 112 Tn, 16 j]
        # -> [p=16*tk8+j, slab, kc, tnl]
        arr = pl.reshape(KC, 8, NSLAB, 16, 16)  # [kc, tk8, slab, tnl, j]
        arr = arr.transpose(1, 4, 2, 0, 3).reshape(128, NSLAB * 512)
        planes[:, c * NSLAB * 512 : (c + 1) * NSLAB * 512] = arr

    # xT[p, kc*8+b] = x[b, kc*128+p]
    xT = np.ascontiguousarray(
        x.reshape(BATCH, KC, 128).transpose(2, 1, 0).reshape(128, KC * BATCH)
    )
    suhT = np.ascontiguousarray(suh.reshape(KC, 128).T)  # [128, 32]

    svh_s = svh[core * NC_COLS : (core + 1) * NC_COLS].astype(np.float32)
    bias_s = bias[core * NC_COLS : (core + 1) * NC_COLS].astype(np.float32)

    return {
        "planes": planes,
        "xT": xT,
        "suhT": suhT,
        "Hmat": _hadamard128(),
        "HP": _perm_h(),
        "ident8": np.eye(8, dtype=np.float32),
        "svhb": np.ascontiguousarray(np.broadcast_to(svh_s, (8, NC_COLS))),
        "biasb": np.ascontiguousarray(np.broadcast_to(bias_s, (8, NC_COLS))),
    }


def kernel(x, trellis, suh, svh, bias):
    x = np.asarray(x)
    trellis = np.asarray(trellis).astype(np.uint16)
    suh = np.asarray(suh)
    svh = np.asarray(svh)
    bias = np.asarray(bias)

    nc = _build_program()
    in_maps = [
        _prep_core_inputs(x, trellis, suh, svh, bias, core) for core in range(NCORES)
    ]
    res = run_bass_kernel_spmd(nc, in_maps, core_ids=list(range(NCORES)))
    global LAST_RUN
    LAST_RUN = res
    out = np.concatenate([res.results[c]["out"] for c in range(NCORES)], axis=1)
    return out.astype(np.float16)


LAST_RUN = None


if __name__ == "__main__":
    import reference as ref
    import jax.numpy as jnp

    inputs = {k: np.asarray(v) for k, v in ref.setup_inputs().items()}
    expected = np.asarray(ref.reference(**{k: jnp.asarray(v) for k, v in inputs.items()}))
    got = kernel(**inputs)
    e = np.linalg.norm(got.astype(np.float32) - expected.astype(np.float32))
    n = np.linalg.norm(expected.astype(np.float32))
    print("Relative error:", e / n)


# revision 12
# speedup vs baseline: 281.2328x; 281.2328x over previous
"""EXL3 trellis-quantized linear layer on 8 Trainium2 NeuronCores.

y = Had(Had(x*suh) @ dequant(trellis)) * svh + bias

Sharding: column-parallel over output features (N). Each of the 8 cores
dequants and multiplies its 1792-column shard; host concatenates.

Decode pipeline per weight (t = column-within-tile class, fixed shift r):
    state = ((A & M1) << r) | (B >> (16-r))     A,B = trellis word pair
    z     = (state*89226354 + 64248484) mod 2^32   (gpsimd int32 TT ops)
    z    &= 0x8FFF8FFF
    w     = fp16(z_lo) + fp16(z_hi)
The fp16 halves are never summed explicitly: the masked z tile is bitcast
to fp16 and streamed into the PE as two rhs streams (lo/hi interleaved)
accumulating into the same PSUM bank.

Weight (j,t) of tile (Tk,Tn) sits at W[16Tk+j, 16Tn+t], so an output
column's weights share one t class. PSUM columns are produced t-major and
the output Hadamard uses a row-permuted H to compensate.
"""

import sys

if "/opt/trn_rl_repo" not in sys.path:
    sys.path.insert(0, "/opt/trn_rl_repo")

import numpy as np

import concourse.bacc as bacc
import concourse.mybir as mybir
from concourse import tile
from concourse.bass_utils import run_bass_kernel_spmd

AL = mybir.AluOpType
DT = mybir.dt

# problem geometry (hardcoded per contest contract)
K = 4096
N = 14336
BATCH = 8
NCORES = 8
TNC = (N // 16) // NCORES  # 112 trellis tile-cols per core
NC_COLS = TNC * 16  # 1792 out features per core
NSLAB = 7  # legacy constant (plane DMA chunking)
SLABS = [(0, 32), (32, 32), (64, 32), (96, 16)]  # (Tn offset, width)
KC = 32  # 128-row k-chunks

CLS_BUFS = 3
LCG_BUFS = 3
LCG_Q = 89226354
LCG_D = 64248484
MASK32 = np.int32(np.uint32(0x8FFF8FFF).astype(np.int64) - (1 << 32))

# per-class constants
CLS = []
for t in range(16):
    c = (3 * t) // 16
    r = 3 * t - 16 * c
    CLS.append((c, r))


def _hadamard128():
    h = np.array([[1.0]], dtype=np.float64)
    while h.shape[0] < 128:
        h = np.block([[h, h], [h, -h]])
    return (h / np.sqrt(128.0)).astype(np.float32)


def _perm_h():
    # psum col f' = t*8 + sub  <->  true in-block col sub*16 + t
    h = _hadamard128()
    pi = np.zeros(128, dtype=np.int64)
    for t in range(16):
        for sub in range(8):
            pi[t * 8 + sub] = sub * 16 + t
    return np.ascontiguousarray(h[pi, :])


_NC_CACHE = {}


def _build_program(variant=""):
    """variant: comma-joined ablation flags for cost attribution:
    nogp (skip LCG), noextract (skip X1/X2/join), nomask, nope (skip MMs)."""
    if variant in _NC_CACHE:
        return _NC_CACHE[variant]
    flags = set(variant.split(",")) if variant else set()

    nc = bacc.Bacc("TRN2", target_bir_lowering=False, debug=False)

    d_planes = nc.dram_tensor("planes", [128, 4 * KC * TNC], DT.uint16, kind="ExternalInput")
    d_xT = nc.dram_tensor("xT", [128, KC * BATCH], DT.float16, kind="ExternalInput")
    d_suhT = nc.dram_tensor("suhT", [128, KC], DT.float16, kind="ExternalInput")
    d_H = nc.dram_tensor("Hmat", [128, 128], DT.float32, kind="ExternalInput")
    d_HP = nc.dram_tensor("HP", [128, 128], DT.float32, kind="ExternalInput")
    d_ident = nc.dram_tensor("ident8", [8, 8], DT.float32, kind="ExternalInput")
    d_svh = nc.dram_tensor("svhb", [8, NC_COLS], DT.float32, kind="ExternalInput")
    d_bias = nc.dram_tensor("biasb", [8, NC_COLS], DT.float32, kind="ExternalInput")
    d_out = nc.dram_tensor("out", [8, NC_COLS], DT.float16, kind="ExternalOutput")

    with tile.TileContext(nc) as tc:
        with (
            tc.tile_pool(name="const", bufs=1) as cpool,
            tc.tile_pool(name="planes", bufs=1) as ppool,
            tc.tile_pool(name="cls", bufs=int(CLS_BUFS)) as clspool,
            tc.tile_pool(name="lcg", bufs=int(LCG_BUFS)) as lcgpool,
            tc.tile_pool(name="zslab", bufs=2) as zpool,
            tc.tile_pool(name="zslab1", bufs=1) as zpool1,
            tc.tile_pool(name="outp", bufs=1) as opool,
            tc.tile_pool(name="psum", bufs=2, space="PSUM") as pspool,
            tc.tile_pool(name="psum_s", bufs=2, space="PSUM") as pspool_s,
        ):
            # ---- constants / small inputs ----
            planes = ppool.tile([128, 4 * KC * TNC], DT.uint16, tag="planes")
            for c4 in range(4):
                sl = slice(c4 * KC * TNC, (c4 + 1) * KC * TNC)
                nc.sync.dma_start(planes[:, sl], d_planes[:, sl])
            t_xT = cpool.tile([128, KC * BATCH], DT.float16, tag="xT")
            t_suhT = cpool.tile([128, KC], DT.float16, tag="suhT")
            t_H = cpool.tile([128, 128], DT.float32, tag="H")
            t_HP = cpool.tile([128, 128], DT.float32, tag="HP")
            t_id8 = cpool.tile([8, 8], DT.float32, tag="id8")
            t_svh = cpool.tile([8, NC_COLS], DT.float32, tag="svh")
            t_bias = cpool.tile([8, NC_COLS], DT.float32, tag="bias")
            nc.sync.dma_start(t_xT[:], d_xT[:])
            nc.sync.dma_start(t_suhT[:], d_suhT[:])
            nc.sync.dma_start(t_H[:], d_H[:])
            nc.sync.dma_start(t_HP[:], d_HP[:])
            nc.sync.dma_start(t_id8[:], d_ident[:])
            nc.sync.dma_start(t_svh[:], d_svh[:])
            nc.sync.dma_start(t_bias[:], d_bias[:])

            t_q = cpool.tile([128, 1], DT.int32, tag="cq")
            t_d = cpool.tile([128, 1], DT.int32, tag="cd")
            t_m = cpool.tile([128, 1], DT.int32, tag="cm")
            nc.vector.memset(t_q[:], LCG_Q)
            nc.vector.memset(t_d[:], LCG_D)
            nc.vector.memset(t_m[:], int(MASK32))

            # ---- input rotation: xhT[j, kc*8+b] ----
            t_xsT = cpool.tile([128, KC * BATCH], DT.float32, tag="xsT")
            nc.vector.tensor_tensor(
                t_xsT[:].rearrange("p (kc b) -> p kc b", kc=KC),
                t_xT[:].rearrange("p (kc b) -> p kc b", kc=KC),
                t_suhT[:].unsqueeze(2).broadcast_to([128, KC, BATCH]),
                AL.mult,
            )
            ps_xh = pspool.tile([128, KC * BATCH], DT.float32, tag="ps_xh")
            nc.tensor.matmul(ps_xh[:], t_H[:], t_xsT[:], start=True, stop=True)
            t_xhT = cpool.tile([128, KC * BATCH], DT.float16, tag="xhT")
            nc.scalar.copy(t_xhT[:], ps_xh[:])

            t_out = opool.tile([8, NC_COLS], DT.float16, tag="outsb")

            # ---- main loop over Tn slabs ----
            for tn0, tnw in SLABS:
                fw = KC * tnw  # class-op free width
                tza = zpool.tile([128, 8 * KC * 32], DT.int32, tag="za")
                tzb = zpool1.tile([128, 8 * KC * 32], DT.int32, tag="zb")
                tzh = [tza, tzb]
                pview = planes[:].rearrange("p (c kc tn) -> p c kc tn", c=4, kc=KC)
                for t16, (c, r) in enumerate(CLS):
                    a_v = pview[:, c, :, tn0 : tn0 + tnw]
                    b_v = pview[:, c + 1, :, tn0 : tn0 + tnw]
                    m1 = (1 << (16 - r)) - 1
                    st_dt = DT.uint16 if "st16" in flags else DT.int32
                    t_st = clspool.tile([128, fw], st_dt, tag="st")
                    if "noextract" in flags:
                        nc.vector.tensor_copy(t_st[:], a_v)
                    else:
                        t_x1 = clspool.tile([128, fw], DT.uint16, tag="x1")
                        t_x2 = clspool.tile([128, fw], DT.uint16, tag="x2")
                        # X1 = (A & M1) << r ; X2 = B >> (16-r)
                        nc.vector.tensor_scalar(
                            t_x1[:], a_v, m1, r, AL.bitwise_and, AL.logical_shift_left
                        )
                        nc.vector.tensor_scalar(
                            t_x2[:], b_v, 16 - r, None, AL.logical_shift_right
                        )
                        # state (disjoint bits: add == or), widen to i32
                        if "joingp" in flags:
                            nc.gpsimd.tensor_tensor(t_st[:], t_x1[:], t_x2[:], AL.add)
                        else:
                            nc.vector.tensor_tensor(t_st[:], t_x1[:], t_x2[:], AL.add)
                    if "nogp" in flags:
                        t_g2 = t_st
                    else:
                        # LCG on gpsimd (exact int32 wraparound)
                        t_g1 = lcgpool.tile([128, fw], DT.int32, tag="g1")
                        t_g2 = lcgpool.tile([128, fw], DT.int32, tag="g2")
                        nc.gpsimd.tensor_tensor(
                            t_g1[:], t_st[:], t_q[:].broadcast_to([128, fw]), AL.mult
                        )
                        nc.gpsimd.tensor_tensor(
                            t_g2[:], t_g1[:], t_d[:].broadcast_to([128, fw]), AL.add
                        )
                    tzv = tzh[t16 // 8][:, (t16 % 8) * fw : (t16 % 8 + 1) * fw]
                    if "nomask" in flags:
                        nc.vector.tensor_copy(tzv, t_g2[:])
                    else:
                        nc.vector.tensor_scalar(
                            tzv, t_g2[:], int(MASK32), None, AL.bitwise_and
                        )

                # ---- matmuls: 2 fp16 streams x 32 k-chunks ----
                nb = tnw // 8  # 128-col blocks in this slab
                ps_y = pspool.tile([8, 512], DT.float32, tag="ps_y")
                ps_yv = ps_y[:, : tnw * 16]
                pv = ps_yv.rearrange("p (b t sub) -> p b t sub", b=nb, t=16, sub=8)
                for half in range(2):
                    zf = tzh[half][:, : 8 * fw].bitcast(DT.float16).rearrange(
                        "p (t kc b sub x) -> p kc x b t sub",
                        t=8, kc=KC, b=nb, sub=8, x=2,
                    )
                    outv = pv[:, :, half * 8 : (half + 1) * 8, :]
                    n_mm = 2 * KC
                    i_mm = 0
                    for xi in range(2):
                        for kc in range(KC):
                            nc.tensor.matmul(
                                outv,
                                t_xhT[:, kc * BATCH : (kc + 1) * BATCH],
                                zf[:, kc, xi],
                                start=(i_mm == 0),
                                stop=(i_mm == n_mm - 1),
                                skip_group_check=True,
                            )
                            i_mm += 1

                # ---- tail: transpose + permuted Hadamard + svh/bias ----
                t_y = clspool.tile([8, 512], DT.float32, tag="ysb")
                nc.scalar.copy(t_y[:, : tnw * 16], ps_yv)
                for bb in range(nb):
                    nblk = (tn0 // 8) + bb
                    ps_t = pspool_s.tile([128, 8], DT.float32, tag="ps_t")
                    nc.tensor.transpose(
                        ps_t[:], t_y[:, bb * 128 : (bb + 1) * 128], t_id8[:]
                    )
                    t_yT = clspool.tile([128, 8], DT.float32, tag="yT")
                    nc.vector.tensor_copy(t_yT[:], ps_t[:])
                    ps_h = pspool_s.tile([8, 128], DT.float32, tag="ps_h")
                    nc.tensor.matmul(ps_h[:], t_yT[:], t_HP[:], start=True, stop=True)
                    t_f = clspool.tile([8, 128], DT.float32, tag="fin")
                    nc.vector.tensor_tensor(
                        t_f[:], ps_h[:], t_svh[:, nblk * 128 : (nblk + 1) * 128], AL.mult
                    )
                    nc.vector.tensor_tensor(
                        t_out[:, nblk * 128 : (nblk + 1) * 128],
                        t_f[:],
                        t_bias[:, nblk * 128 : (nblk + 1) * 128],
                        AL.add,
                    )

            nc.sync.dma_start(d_out[:], t_out[:])

    nc.compile()
    _NC_CACHE[variant] = nc
    return nc


def _prep_core_inputs(x, trellis, suh, svh, bias, core):
    tshard = trellis[:, core * TNC : (core + 1) * TNC, :]  # [256, 112, 48]
    j = np.arange(16)
    planes = np.empty((128, 4 * KC * TNC), dtype=np.uint16)
    for c in range(4):
        w = (3 * j + c) % 48
        pl = tshard[:, :, w]  # [256 Tk, 112 Tn, 16 j]
        # -> [p=16*tk8+j, kc, Tn]
        arr = pl.reshape(KC, 8, TNC, 16)  # [kc, tk8, Tn, j]
        arr = arr.transpose(1, 3, 0, 2).reshape(128, KC * TNC)
        planes[:, c * KC * TNC : (c + 1) * KC * TNC] = arr

    # xT[p, kc*8+b] = x[b, kc*128+p]
    xT = np.ascontiguousarray(
        x.reshape(BATCH, KC, 128).transpose(2, 1, 0).reshape(128, KC * BATCH)
    )
    suhT = np.ascontiguousarray(suh.reshape(KC, 128).T)  # [128, 32]

    svh_s = svh[core * NC_COLS : (core + 1) * NC_COLS].astype(np.float32)
    bias_s = bias[core * NC_COLS : (core + 1) * NC_COLS].astype(np.float32)

    return {
        "planes": planes,
        "xT": xT,
        "suhT": suhT,
        "Hmat": _hadamard128(),
        "HP": _perm_h(),
        "ident8": np.eye(8, dtype=np.float32),
        "svhb": np.ascontiguousarray(np.broadcast_to(svh_s, (8, NC_COLS))),
        "biasb": np.ascontiguousarray(np.broadcast_to(bias_s, (8, NC_COLS))),
    }


def kernel(x, trellis, suh, svh, bias):
    x = np.asarray(x)
    trellis = np.asarray(trellis).astype(np.uint16)
    suh = np.asarray(suh)
    svh = np.asarray(svh)
    bias = np.asarray(bias)

    nc = _build_program()
    in_maps = [
        _prep_core_inputs(x, trellis, suh, svh, bias, core) for core in range(NCORES)
    ]
    res = run_bass_kernel_spmd(nc, in_maps, core_ids=list(range(NCORES)))
    global LAST_RUN
    LAST_RUN = res
    out = np.concatenate([res.results[c]["out"] for c in range(NCORES)], axis=1)
    return out.astype(np.float16)


LAST_RUN = None


if __name__ == "__main__":
    import reference as ref
    import jax.numpy as jnp

    inputs = {k: np.asarray(v) for k, v in ref.setup_inputs().items()}
    expected = np.asarray(ref.reference(**{k: jnp.asarray(v) for k, v in inputs.items()}))
    got = kernel(**inputs)
    e = np.linalg.norm(got.astype(np.float32) - expected.astype(np.float32))
    n = np.linalg.norm(expected.astype(np.float32))
    print("Relative error:", e / n)


# revision 19
# speedup vs baseline: 307.2980x; 1.0927x over previous
"""EXL3 trellis-quantized linear layer on 8 Trainium2 NeuronCores.

y = Had(Had(x*suh) @ dequant(trellis)) * svh + bias

Sharding: column-parallel over output features (N). Each of the 8 cores
dequants and multiplies its 1792-column shard; host concatenates.

Decode pipeline per weight (t = column-within-tile class, fixed shift r):
    state = ((A & M1) << r) | (B >> (16-r))     A,B = trellis word pair
    z     = (state*89226354 + 64248484) mod 2^32   (gpsimd int32 TT ops)
    z    &= 0x8FFF8FFF
    w     = fp16(z_lo) + fp16(z_hi)
The fp16 halves are never summed explicitly: the masked z tile is bitcast
to fp16 and streamed into the PE as two rhs streams (lo/hi interleaved)
accumulating into the same PSUM bank.

Weight (j,t) of tile (Tk,Tn) sits at W[16Tk+j, 16Tn+t], so an output
column's weights share one t class. PSUM columns are produced t-major and
the output Hadamard uses a row-permuted H to compensate.
"""

import sys

if "/opt/trn_rl_repo" not in sys.path:
    sys.path.insert(0, "/opt/trn_rl_repo")

import numpy as np

import concourse.bacc as bacc
import concourse.mybir as mybir
from concourse import tile
from concourse.bass_utils import run_bass_kernel_spmd

AL = mybir.AluOpType
DT = mybir.dt

# problem geometry (hardcoded per contest contract)
K = 4096
N = 14336
BATCH = 8
NCORES = 8
TNC = (N // 16) // NCORES  # 112 trellis tile-cols per core
NC_COLS = TNC * 16  # 1792 out features per core
NSLAB = 7  # legacy constant (plane DMA chunking)
SLABS = [(0, 32), (32, 32), (64, 32), (96, 16)]  # (Tn offset, width)
KC = 32  # 128-row k-chunks

CLS_BUFS = 3
LCG_BUFS = 3
LCG_Q = 89226354
LCG_D = 64248484
DELTA16 = 14306  # delta*Q ≡ D (mod 2^16)
RHO16 = 53288  # (D - DELTA16*Q) >> 16 (mod 2^32)
MASK32 = np.int32(np.uint32(0x8FFF8FFF).astype(np.int64) - (1 << 32))

# per-class constants
CLS = []
for t in range(16):
    c = (3 * t) // 16
    r = 3 * t - 16 * c
    CLS.append((c, r))


def _hadamard128():
    h = np.array([[1.0]], dtype=np.float64)
    while h.shape[0] < 128:
        h = np.block([[h, h], [h, -h]])
    return (h / np.sqrt(128.0)).astype(np.float32)


def _perm_h():
    # psum col f' = t*8 + sub  <->  true in-block col sub*16 + t
    h = _hadamard128()
    pi = np.zeros(128, dtype=np.int64)
    for t in range(16):
        for sub in range(8):
            pi[t * 8 + sub] = sub * 16 + t
    return np.ascontiguousarray(h[pi, :])


_NC_CACHE = {}


def _build_program(variant=""):
    """variant: comma-joined ablation flags for cost attribution:
    nogp (skip LCG), noextract (skip X1/X2/join), nomask, nope (skip MMs)."""
    if variant in _NC_CACHE:
        return _NC_CACHE[variant]
    flags = set(variant.split(",")) if variant else set()

    nc = bacc.Bacc("TRN2", target_bir_lowering=False, debug=False)

    d_planes = nc.dram_tensor("planes", [128, 4 * KC * TNC], DT.uint16, kind="ExternalInput")
    d_xT = nc.dram_tensor("xT", [128, KC * BATCH], DT.float16, kind="ExternalInput")
    d_suhT = nc.dram_tensor("suhT", [128, KC], DT.float16, kind="ExternalInput")
    d_H = nc.dram_tensor("Hmat", [128, 128], DT.float32, kind="ExternalInput")
    d_HP = nc.dram_tensor("HP", [128, 128], DT.float32, kind="ExternalInput")
    d_ident = nc.dram_tensor("ident8", [8, 8], DT.float32, kind="ExternalInput")
    d_svh = nc.dram_tensor("svhb", [8, NC_COLS], DT.float32, kind="ExternalInput")
    d_bias = nc.dram_tensor("biasb", [8, NC_COLS], DT.float32, kind="ExternalInput")
    d_out = nc.dram_tensor("out", [8, NC_COLS], DT.float16, kind="ExternalOutput")

    with tile.TileContext(nc) as tc:
        with (
            tc.tile_pool(name="const", bufs=1) as cpool,
            tc.tile_pool(name="planes", bufs=1) as ppool,
            tc.tile_pool(name="cls", bufs=int(CLS_BUFS)) as clspool,
            tc.tile_pool(name="lcg", bufs=int(LCG_BUFS)) as lcgpool,
            tc.tile_pool(name="zslab", bufs=2) as zpool,
            tc.tile_pool(name="zslab1", bufs=1) as zpool1,
            tc.tile_pool(name="outp", bufs=1) as opool,
            tc.tile_pool(name="psum", bufs=2, space="PSUM") as pspool,
            tc.tile_pool(name="psum_s", bufs=2, space="PSUM") as pspool_s,
        ):
            # ---- constants / small inputs ----
            planes = ppool.tile([128, 4 * KC * TNC], DT.uint16, tag="planes")
            for c4 in range(4):
                sl = slice(c4 * KC * TNC, (c4 + 1) * KC * TNC)
                nc.sync.dma_start(planes[:, sl], d_planes[:, sl])
            t_xT = cpool.tile([128, KC * BATCH], DT.float16, tag="xT")
            t_suhT = cpool.tile([128, KC], DT.float16, tag="suhT")
            t_H = cpool.tile([128, 128], DT.float32, tag="H")
            t_HP = cpool.tile([128, 128], DT.float32, tag="HP")
            t_id8 = cpool.tile([8, 8], DT.float32, tag="id8")
            t_svh = cpool.tile([8, NC_COLS], DT.float32, tag="svh")
            t_bias = cpool.tile([8, NC_COLS], DT.float32, tag="bias")
            nc.sync.dma_start(t_xT[:], d_xT[:])
            nc.sync.dma_start(t_suhT[:], d_suhT[:])
            nc.sync.dma_start(t_H[:], d_H[:])
            nc.sync.dma_start(t_HP[:], d_HP[:])
            nc.sync.dma_start(t_id8[:], d_ident[:])
            nc.sync.dma_start(t_svh[:], d_svh[:])
            nc.sync.dma_start(t_bias[:], d_bias[:])

            t_q = cpool.tile([128, 1], DT.int32, tag="cq")
            t_d = cpool.tile([128, 1], DT.int32, tag="cd")
            t_m = cpool.tile([128, 1], DT.int32, tag="cm")
            nc.vector.memset(t_q[:], LCG_Q)
            nc.vector.memset(t_d[:], LCG_D)
            nc.vector.memset(t_m[:], int(MASK32))
            t_rho = cpool.tile([128, 1], DT.float32, tag="crho")
            nc.vector.memset(t_rho[:], float(RHO16))
            t_m16 = cpool.tile([128, 1], DT.int16, tag="cm16")
            nc.vector.memset(t_m16[:], 0x8FFF - (1 << 16) + (1 << 16) if False else 0x0FFF | 0x8000 - 0x10000)

            # ---- input rotation: xhT[j, kc*8+b] ----
            t_xsT = cpool.tile([128, KC * BATCH], DT.float32, tag="xsT")
            nc.vector.tensor_tensor(
                t_xsT[:].rearrange("p (kc b) -> p kc b", kc=KC),
                t_xT[:].rearrange("p (kc b) -> p kc b", kc=KC),
                t_suhT[:].unsqueeze(2).broadcast_to([128, KC, BATCH]),
                AL.mult,
            )
            ps_xh = pspool.tile([128, KC * BATCH], DT.float32, tag="ps_xh")
            nc.tensor.matmul(ps_xh[:], t_H[:], t_xsT[:], start=True, stop=True)
            t_xhT = cpool.tile([128, KC * BATCH], DT.float16, tag="xhT")
            nc.scalar.copy(t_xhT[:], ps_xh[:])

            t_out = opool.tile([8, NC_COLS], DT.float16, tag="outsb")

            # ---- main loop over Tn slabs ----
            for tn0, tnw in SLABS:
                fw = KC * tnw  # class-op free width
                tza = zpool.tile([128, 8 * KC * 32], DT.int32, tag="za")
                tzb = zpool1.tile([128, 8 * KC * 32], DT.int32, tag="zb")
                tzh = [tza, tzb]
                pview = planes[:].rearrange("p (c kc tn) -> p c kc tn", c=4, kc=KC)
                for t16, (c, r) in enumerate(CLS):
                    a_v = pview[:, c, :, tn0 : tn0 + tnw]
                    b_v = pview[:, c + 1, :, tn0 : tn0 + tnw]
                    m1 = (1 << (16 - r)) - 1
                    st_dt = DT.uint16 if "st16" in flags else DT.int32
                    t_st = clspool.tile([128, fw], st_dt, tag="st")
                    if "noextract" in flags:
                        nc.vector.tensor_copy(t_st[:], a_v)
                    else:
                        t_x1 = clspool.tile([128, fw], DT.uint16, tag="x1")
                        t_x2 = clspool.tile([128, fw], DT.uint16, tag="x2")
                        # X1 = (A & M1) << r ; X2 = B >> (16-r)
                        nc.vector.tensor_scalar(
                            t_x1[:], a_v, m1, r, AL.bitwise_and, AL.logical_shift_left
                        )
                        nc.vector.tensor_scalar(
                            t_x2[:], b_v, 16 - r, None, AL.logical_shift_right
                        )
                        # state+delta (bits disjoint; +delta folds the LCG
                        # offset: z = (st+delta)*Q + rho*2^16), widen to i32
                        nc.vector.scalar_tensor_tensor(
                            t_st[:], t_x1[:], float(DELTA16), t_x2[:],
                            op0=AL.add, op1=AL.add,
                        )
                    if "nogp" in flags:
                        t_g1 = t_st
                    else:
                        # LCG multiply on gpsimd (exact int32 wraparound)
                        t_g1 = lcgpool.tile([128, fw], DT.int32, tag="g1")
                        nc.gpsimd.tensor_tensor(
                            t_g1[:], t_st[:], t_q[:].broadcast_to([128, fw]), AL.mult
                        )
                    tzv = tzh[t16 // 8][:, (t16 % 8) * fw : (t16 % 8 + 1) * fw]
                    if "nomask" in flags:
                        nc.vector.tensor_copy(tzv, t_g1[:])
                    else:
                        nc.vector.tensor_scalar(
                            tzv, t_g1[:], int(MASK32), None, AL.bitwise_and
                        )
                        # hi halves need +rho (mod 2^16) before masking:
                        # ACT does the exact add on the odd int16 view,
                        # DVE masks and writes the odd halves back.
                        t_h32 = lcgpool.tile([128, fw], DT.int32, tag="h32")
                        zq_odd = t_g1[:].bitcast(DT.int16).rearrange(
                            "p (n x) -> p x n", x=2
                        )[:, 1]
                        nc.scalar.activation(
                            t_h32[:], zq_odd,
                            mybir.ActivationFunctionType.Identity,
                            bias=t_rho[:], scale=1.0,
                        )
                        tz_odd = tzv.bitcast(DT.int16).rearrange(
                            "p (n x) -> p x n", x=2
                        )[:, 1]
                        h32_lo = t_h32[:].bitcast(DT.int16).rearrange(
                            "p (n x) -> p x n", x=2
                        )[:, 0]
                        if "oddgp" in flags:
                            nc.gpsimd.tensor_tensor(
                                tz_odd, h32_lo,
                                t_m16[:].broadcast_to([128, fw]), AL.bitwise_and,
                            )
                        else:
                            nc.vector.tensor_scalar(
                                tz_odd, h32_lo, 0x8FFF, None, AL.bitwise_and
                            )

                # ---- matmuls: 2 fp16 streams x 32 k-chunks ----
                nb = tnw // 8  # 128-col blocks in this slab
                ps_y = pspool.tile([8, 512], DT.float32, tag="ps_y")
                ps_yv = ps_y[:, : tnw * 16]
                pv = ps_yv.rearrange("p (b t sub) -> p b t sub", b=nb, t=16, sub=8)
                for half in range(2):
                    zf = tzh[half][:, : 8 * fw].bitcast(DT.float16).rearrange(
                        "p (t kc b sub x) -> p kc x b t sub",
                        t=8, kc=KC, b=nb, sub=8, x=2,
                    )
                    outv = pv[:, :, half * 8 : (half + 1) * 8, :]
                    n_mm = 2 * KC
                    i_mm = 0
                    for xi in range(2):
                        for kc in range(KC):
                            nc.tensor.matmul(
                                outv,
                                t_xhT[:, kc * BATCH : (kc + 1) * BATCH],
                                zf[:, kc, xi],
                                start=(i_mm == 0),
                                stop=(i_mm == n_mm - 1),
                                skip_group_check=True,
                            )
                            i_mm += 1

                # ---- tail: transpose + permuted Hadamard + svh/bias ----
                t_y = clspool.tile([8, 512], DT.float32, tag="ysb")
                nc.scalar.copy(t_y[:, : tnw * 16], ps_yv)
                for bb in range(nb):
                    nblk = (tn0 // 8) + bb
                    ps_t = pspool_s.tile([128, 8], DT.float32, tag="ps_t")
                    nc.tensor.transpose(
                        ps_t[:], t_y[:, bb * 128 : (bb + 1) * 128], t_id8[:]
                    )
                    t_yT = clspool.tile([128, 8], DT.float32, tag="yT")
                    nc.vector.tensor_copy(t_yT[:], ps_t[:])
                    ps_h = pspool_s.tile([8, 128], DT.float32, tag="ps_h")
                    nc.tensor.matmul(ps_h[:], t_yT[:], t_HP[:], start=True, stop=True)
                    t_f = clspool.tile([8, 128], DT.float32, tag="fin")
                    nc.vector.tensor_tensor(
                        t_f[:], ps_h[:], t_svh[:, nblk * 128 : (nblk + 1) * 128], AL.mult
                    )
                    nc.vector.tensor_tensor(
                        t_out[:, nblk * 128 : (nblk + 1) * 128],
                        t_f[:],
                        t_bias[:, nblk * 128 : (nblk + 1) * 128],
                        AL.add,
                    )

            nc.sync.dma_start(d_out[:], t_out[:])

    nc.compile()
    _NC_CACHE[variant] = nc
    return nc


def _prep_core_inputs(x, trellis, suh, svh, bias, core):
    tshard = trellis[:, core * TNC : (core + 1) * TNC, :]  # [256, 112, 48]
    j = np.arange(16)
    planes = np.empty((128, 4 * KC * TNC), dtype=np.uint16)
    for c in range(4):
        w = (3 * j + c) % 48
        pl = tshard[:, :, w]  # [256 Tk, 112 Tn, 16 j]
        # -> [p=16*tk8+j, kc, Tn]
        arr = pl.reshape(KC, 8, TNC, 16)  # [kc, tk8, Tn, j]
        arr = arr.transpose(1, 3, 0, 2).reshape(128, KC * TNC)
        planes[:, c * KC * TNC : (c + 1) * KC * TNC] = arr

    # xT[p, kc*8+b] = x[b, kc*128+p]
    xT = np.ascontiguousarray(
        x.reshape(BATCH, KC, 128).transpose(2, 1, 0).reshape(128, KC * BATCH)
    )
    suhT = np.ascontiguousarray(suh.reshape(KC, 128).T)  # [128, 32]

    svh_s = svh[core * NC_COLS : (core + 1) * NC_COLS].astype(np.float32)
    bias_s = bias[core * NC_COLS : (core + 1) * NC_COLS].astype(np.float32)

    return {
        "planes": planes,
        "xT": xT,
        "suhT": suhT,
        "Hmat": _hadamard128(),
        "HP": _perm_h(),
        "ident8": np.eye(8, dtype=np.float32),
        "svhb": np.ascontiguousarray(np.broadcast_to(svh_s, (8, NC_COLS))),
        "biasb": np.ascontiguousarray(np.broadcast_to(bias_s, (8, NC_COLS))),
    }


def kernel(x, trellis, suh, svh, bias):
    x = np.asarray(x)
    trellis = np.asarray(trellis).astype(np.uint16)
    suh = np.asarray(suh)
    svh = np.asarray(svh)
    bias = np.asarray(bias)

    nc = _build_program()
    in_maps = [
        _prep_core_inputs(x, trellis, suh, svh, bias, core) for core in range(NCORES)
    ]
    res = run_bass_kernel_spmd(nc, in_maps, core_ids=list(range(NCORES)))
    global LAST_RUN
    LAST_RUN = res
    out = np.concatenate([res.results[c]["out"] for c in range(NCORES)], axis=1)
    return out.astype(np.float16)


LAST_RUN = None


if __name__ == "__main__":
    import reference as ref
    import jax.numpy as jnp

    inputs = {k: np.asarray(v) for k, v in ref.setup_inputs().items()}
    expected = np.asarray(ref.reference(**{k: jnp.asarray(v) for k, v in inputs.items()}))
    got = kernel(**inputs)
    e = np.linalg.norm(got.astype(np.float32) - expected.astype(np.float32))
    n = np.linalg.norm(expected.astype(np.float32))
    print("Relative error:", e / n)


# revision 22
# speedup vs baseline: 317.6725x; 1.0338x over previous
"""EXL3 trellis-quantized linear layer on 8 Trainium2 NeuronCores.

y = Had(Had(x*suh) @ dequant(trellis)) * svh + bias

Sharding: column-parallel over output features (N). Each of the 8 cores
dequants and multiplies its 1792-column shard; host concatenates.

Decode pipeline per weight (t = column-within-tile class, fixed shift r):
    state = ((A & M1) << r) | (B >> (16-r))      A,B = trellis word pair
    z     = (state*Q + D) mod 2^32;  z &= 0x8FFF8FFF
    w     = fp16(z_lo) + fp16(z_hi)
Engine split: extraction on DVE (fused and+shl / shr tensor_scalars, join
via STT which also adds delta = D*Q^-1 mod 2^16 so the LCG needs only ONE
gpsimd int32 multiply: z = (state+delta)*Q + rho*2^16). The rho correction
touches only the hi int16 halves: ACT adds rho on the odd int16 view and
DVE masks/writes the odd halves back. The fp16 halves are never summed
explicitly: the masked z tile is bitcast to fp16 and streamed to the PE as
two rhs streams (lo/hi interleaved) accumulating into the same PSUM bank.

Weight (j,t) of tile (Tk,Tn) sits at W[16Tk+j, 16Tn+t], so an output
column's weights share one t class. PSUM columns are produced t-major and
the output Hadamard uses a row-permuted H to compensate.
"""

import sys

if "/opt/trn_rl_repo" not in sys.path:
    sys.path.insert(0, "/opt/trn_rl_repo")

import numpy as np

import concourse.bacc as bacc
import concourse.mybir as mybir
from concourse import tile
from concourse.bass_utils import run_bass_kernel_spmd

AL = mybir.AluOpType
DT = mybir.dt

# problem geometry (hardcoded per contest contract)
K = 4096
N = 14336
BATCH = 8
NCORES = 8
TNC = (N // 16) // NCORES  # 112 trellis tile-cols per core
NC_COLS = TNC * 16  # 1792 out features per core
NSLAB = 7  # legacy constant (plane DMA chunking)
SLABS = [(0, 32), (32, 32), (64, 32), (96, 16)]  # (Tn offset, width)
KC = 32  # 128-row k-chunks

CLS_BUFS = 3
LCG_BUFS = 3
LCG_Q = 89226354
LCG_D = 64248484
DELTA16 = 14306  # delta*Q ≡ D (mod 2^16)
RHO16 = 53288  # (D - DELTA16*Q) >> 16 (mod 2^32)
MASK32 = np.int32(np.uint32(0x8FFF8FFF).astype(np.int64) - (1 << 32))

# per-class constants
CLS = []
for t in range(16):
    c = (3 * t) // 16
    r = 3 * t - 16 * c
    CLS.append((c, r))


def _hadamard128():
    h = np.array([[1.0]], dtype=np.float64)
    while h.shape[0] < 128:
        h = np.block([[h, h], [h, -h]])
    return (h / np.sqrt(128.0)).astype(np.float32)


def _perm_h():
    # psum col f' = t*8 + sub  <->  true in-block col sub*16 + t
    h = _hadamard128()
    pi = np.zeros(128, dtype=np.int64)
    for t in range(16):
        for sub in range(8):
            pi[t * 8 + sub] = sub * 16 + t
    return np.ascontiguousarray(h[pi, :])


_NC_CACHE = {}


def _build_program(variant=""):
    """variant: comma-joined ablation flags for cost attribution:
    nogp (skip LCG), noextract (skip X1/X2/join), nomask, nope (skip MMs)."""
    if variant in _NC_CACHE:
        return _NC_CACHE[variant]
    flags = set(variant.split(",")) if variant else set()

    nc = bacc.Bacc("TRN2", target_bir_lowering=False, debug=False)

    d_planes = nc.dram_tensor("planes", [128, 4 * KC * TNC], DT.uint16, kind="ExternalInput")
    d_xT = nc.dram_tensor("xT", [128, KC * BATCH], DT.float16, kind="ExternalInput")
    d_suhT = nc.dram_tensor("suhT", [128, KC], DT.float16, kind="ExternalInput")
    d_H = nc.dram_tensor("Hmat", [128, 128], DT.float32, kind="ExternalInput")
    d_HP = nc.dram_tensor("HP", [128, 128], DT.float32, kind="ExternalInput")
    d_ident = nc.dram_tensor("ident8", [8, 8], DT.float32, kind="ExternalInput")
    d_svh = nc.dram_tensor("svhb", [8, NC_COLS], DT.float32, kind="ExternalInput")
    d_bias = nc.dram_tensor("biasb", [8, NC_COLS], DT.float32, kind="ExternalInput")
    d_out = nc.dram_tensor("out", [8, NC_COLS], DT.float16, kind="ExternalOutput")

    with tile.TileContext(nc) as tc:
        with (
            tc.tile_pool(name="const", bufs=1) as cpool,
            tc.tile_pool(name="planes", bufs=1) as ppool,
            tc.tile_pool(name="cls", bufs=int(CLS_BUFS)) as clspool,
            tc.tile_pool(name="lcg", bufs=int(LCG_BUFS)) as lcgpool,
            tc.tile_pool(name="zslab", bufs=2) as zpool,
            tc.tile_pool(name="zslab1", bufs=1) as zpool1,
            tc.tile_pool(name="outp", bufs=1) as opool,
            tc.tile_pool(name="psum", bufs=2, space="PSUM") as pspool,
            tc.tile_pool(name="psum_s", bufs=2, space="PSUM") as pspool_s,
        ):
            # ---- constants / small inputs ----
            planes = ppool.tile([128, 4 * KC * TNC], DT.uint16, tag="planes")
            for c4 in range(4):
                sl = slice(c4 * KC * TNC, (c4 + 1) * KC * TNC)
                nc.sync.dma_start(planes[:, sl], d_planes[:, sl])
            t_xT = cpool.tile([128, KC * BATCH], DT.float16, tag="xT")
            t_suhT = cpool.tile([128, KC], DT.float16, tag="suhT")
            t_H = cpool.tile([128, 128], DT.float32, tag="H")
            t_HP = cpool.tile([128, 128], DT.float32, tag="HP")
            t_id8 = cpool.tile([8, 8], DT.float32, tag="id8")
            t_svh = cpool.tile([8, NC_COLS], DT.float32, tag="svh")
            t_bias = cpool.tile([8, NC_COLS], DT.float32, tag="bias")
            nc.sync.dma_start(t_xT[:], d_xT[:])
            nc.sync.dma_start(t_suhT[:], d_suhT[:])
            nc.sync.dma_start(t_H[:], d_H[:])
            nc.sync.dma_start(t_HP[:], d_HP[:])
            nc.sync.dma_start(t_id8[:], d_ident[:])
            nc.sync.dma_start(t_svh[:], d_svh[:])
            nc.sync.dma_start(t_bias[:], d_bias[:])

            t_q = cpool.tile([128, 1], DT.int32, tag="cq")
            nc.vector.memset(t_q[:], LCG_Q)
            t_rho = cpool.tile([128, 1], DT.float32, tag="crho")
            nc.vector.memset(t_rho[:], float(RHO16))

            # ---- input rotation: xhT[j, kc*8+b] ----
            t_xsT = cpool.tile([128, KC * BATCH], DT.float32, tag="xsT")
            nc.vector.tensor_tensor(
                t_xsT[:].rearrange("p (kc b) -> p kc b", kc=KC),
                t_xT[:].rearrange("p (kc b) -> p kc b", kc=KC),
                t_suhT[:].unsqueeze(2).broadcast_to([128, KC, BATCH]),
                AL.mult,
            )
            ps_xh = pspool.tile([128, KC * BATCH], DT.float32, tag="ps_xh")
            nc.tensor.matmul(ps_xh[:], t_H[:], t_xsT[:], start=True, stop=True)
            t_xhT = cpool.tile([128, KC * BATCH], DT.float16, tag="xhT")
            nc.scalar.copy(t_xhT[:], ps_xh[:])

            t_out = opool.tile([8, NC_COLS], DT.float16, tag="outsb")

            # ---- main loop over Tn slabs ----
            for tn0, tnw in SLABS:
                fw = KC * tnw  # class-op free width
                tza = zpool.tile([128, 8 * KC * 32], DT.int32, tag="za")
                tzb = zpool1.tile([128, 8 * KC * 32], DT.int32, tag="zb")
                tzh = [tza, tzb]
                pview = planes[:].rearrange("p (c kc tn) -> p c kc tn", c=4, kc=KC)
                for t16, (c, r) in enumerate(CLS):
                    a_v = pview[:, c, :, tn0 : tn0 + tnw]
                    b_v = pview[:, c + 1, :, tn0 : tn0 + tnw]
                    m1 = (1 << (16 - r)) - 1
                    st_dt = DT.uint16 if "st16" in flags else DT.int32
                    t_st = clspool.tile([128, fw], st_dt, tag="st")
                    if "noextract" in flags:
                        nc.vector.tensor_copy(t_st[:], a_v)
                    elif "nospec" not in flags and r == 0:
                        # state = A; one fused add-delta widening op
                        nc.vector.tensor_scalar(
                            t_st[:], a_v, float(DELTA16), None, AL.add
                        )
                    elif "nospec" not in flags and r == 8:
                        # X2 = B>>8 is just B's high byte: free u8 view
                        t_x1 = clspool.tile([128, fw], DT.uint16, tag="x1")
                        nc.vector.tensor_scalar(
                            t_x1[:], a_v, m1, r, AL.bitwise_and, AL.logical_shift_left
                        )
                        b_hi = planes[:].bitcast(DT.uint8).rearrange(
                            "p (c kc tn x) -> p c kc tn x", c=4, kc=KC, x=2
                        )[:, c + 1, :, tn0 : tn0 + tnw, 1]
                        nc.vector.scalar_tensor_tensor(
                            t_st[:], t_x1[:], float(DELTA16), b_hi,
                            op0=AL.add, op1=AL.add,
                        )
                    else:
                        t_x1 = clspool.tile([128, fw], DT.uint16, tag="x1")
                        t_x2 = clspool.tile([128, fw], DT.uint16, tag="x2")
                        # X1 = (A & M1) << r ; X2 = B >> (16-r)
                        nc.vector.tensor_scalar(
                            t_x1[:], a_v, m1, r, AL.bitwise_and, AL.logical_shift_left
                        )
                        nc.vector.tensor_scalar(
                            t_x2[:], b_v, 16 - r, None, AL.logical_shift_right
                        )
                        # state+delta (bits disjoint; +delta folds the LCG
                        # offset: z = (st+delta)*Q + rho*2^16), widen to i32
                        nc.vector.scalar_tensor_tensor(
                            t_st[:], t_x1[:], float(DELTA16), t_x2[:],
                            op0=AL.add, op1=AL.add,
                        )
                    if "nogp" in flags:
                        t_g1 = t_st
                    else:
                        # LCG multiply on gpsimd (exact int32 wraparound)
                        t_g1 = lcgpool.tile([128, fw], DT.int32, tag="g1")
                        nc.gpsimd.tensor_tensor(
                            t_g1[:], t_st[:], t_q[:].broadcast_to([128, fw]), AL.mult
                        )
                    tzv = tzh[t16 // 8][:, (t16 % 8) * fw : (t16 % 8 + 1) * fw]
                    if "nomask" in flags:
                        nc.vector.tensor_copy(tzv, t_g1[:])
                    else:
                        nc.vector.tensor_scalar(
                            tzv, t_g1[:], int(MASK32), None, AL.bitwise_and
                        )
                        # hi halves need +rho (mod 2^16) before masking:
                        # ACT does the exact add on the odd int16 view,
                        # DVE masks and writes the odd halves back.
                        t_h32 = lcgpool.tile([128, fw], DT.int32, tag="h32")
                        zq_odd = t_g1[:].bitcast(DT.int16).rearrange(
                            "p (n x) -> p x n", x=2
                        )[:, 1]
                        nc.scalar.activation(
                            t_h32[:], zq_odd,
                            mybir.ActivationFunctionType.Identity,
                            bias=t_rho[:], scale=1.0,
                        )
                        tz_odd = tzv.bitcast(DT.int16).rearrange(
                            "p (n x) -> p x n", x=2
                        )[:, 1]
                        h32_lo = t_h32[:].bitcast(DT.int16).rearrange(
                            "p (n x) -> p x n", x=2
                        )[:, 0]
                        nc.vector.tensor_scalar(
                            tz_odd, h32_lo, 0x8FFF, None, AL.bitwise_and
                        )

                # ---- matmuls: 2 fp16 streams x 32 k-chunks ----
                nb = tnw // 8  # 128-col blocks in this slab
                ps_y = pspool.tile([8, 512], DT.float32, tag="ps_y")
                ps_yv = ps_y[:, : tnw * 16]
                pv = ps_yv.rearrange("p (b t sub) -> p b t sub", b=nb, t=16, sub=8)
                for half in range(2):
                    zf = tzh[half][:, : 8 * fw].bitcast(DT.float16).rearrange(
                        "p (t kc b sub x) -> p kc x b t sub",
                        t=8, kc=KC, b=nb, sub=8, x=2,
                    )
                    outv = pv[:, :, half * 8 : (half + 1) * 8, :]
                    n_mm = 2 * KC
                    i_mm = 0
                    for xi in range(2):
                        for kc in range(KC):
                            nc.tensor.matmul(
                                outv,
                                t_xhT[:, kc * BATCH : (kc + 1) * BATCH],
                                zf[:, kc, xi],
                                start=(i_mm == 0),
                                stop=(i_mm == n_mm - 1),
                                skip_group_check=True,
                            )
                            i_mm += 1

                # ---- tail: transpose + permuted Hadamard + svh/bias ----
                t_y = clspool.tile([8, 512], DT.float32, tag="ysb")
                nc.scalar.copy(t_y[:, : tnw * 16], ps_yv)
                for bb in range(nb):
                    nblk = (tn0 // 8) + bb
                    ps_t = pspool_s.tile([128, 8], DT.float32, tag="ps_t")
                    nc.tensor.transpose(
                        ps_t[:], t_y[:, bb * 128 : (bb + 1) * 128], t_id8[:]
                    )
                    t_yT = clspool.tile([128, 8], DT.float32, tag="yT")
                    nc.vector.tensor_copy(t_yT[:], ps_t[:])
                    ps_h = pspool_s.tile([8, 128], DT.float32, tag="ps_h")
                    nc.tensor.matmul(ps_h[:], t_yT[:], t_HP[:], start=True, stop=True)
                    t_f = clspool.tile([8, 128], DT.float32, tag="fin")
                    nc.vector.tensor_tensor(
                        t_f[:], ps_h[:], t_svh[:, nblk * 128 : (nblk + 1) * 128], AL.mult
                    )
                    nc.vector.tensor_tensor(
                        t_out[:, nblk * 128 : (nblk + 1) * 128],
                        t_f[:],
                        t_bias[:, nblk * 128 : (nblk + 1) * 128],
                        AL.add,
                    )

            nc.sync.dma_start(d_out[:], t_out[:])

    nc.compile()
    _NC_CACHE[variant] = nc
    return nc


def _prep_core_inputs(x, trellis, suh, svh, bias, core):
    tshard = trellis[:, core * TNC : (core + 1) * TNC, :]  # [256, 112, 48]
    j = np.arange(16)
    planes = np.empty((128, 4 * KC * TNC), dtype=np.uint16)
    for c in range(4):
        w = (3 * j + c) % 48
        pl = tshard[:, :, w]  # [256 Tk, 112 Tn, 16 j]
        # -> [p=16*tk8+j, kc, Tn]
        arr = pl.reshape(KC, 8, TNC, 16)  # [kc, tk8, Tn, j]
        arr = arr.transpose(1, 3, 0, 2).reshape(128, KC * TNC)
        planes[:, c * KC * TNC : (c + 1) * KC * TNC] = arr

    # xT[p, kc*8+b] = x[b, kc*128+p]
    xT = np.ascontiguousarray(
        x.reshape(BATCH, KC, 128).transpose(2, 1, 0).reshape(128, KC * BATCH)
    )
    suhT = np.ascontiguousarray(suh.reshape(KC, 128).T)  # [128, 32]

    svh_s = svh[core * NC_COLS : (core + 1) * NC_COLS].astype(np.float32)
    bias_s = bias[core * NC_COLS : (core + 1) * NC_COLS].astype(np.float32)

    return {
        "planes": planes,
        "xT": xT,
        "suhT": suhT,
        "Hmat": _hadamard128(),
        "HP": _perm_h(),
        "ident8": np.eye(8, dtype=np.float32),
        "svhb": np.ascontiguousarray(np.broadcast_to(svh_s, (8, NC_COLS))),
        "biasb": np.ascontiguousarray(np.broadcast_to(bias_s, (8, NC_COLS))),
    }


def kernel(x, trellis, suh, svh, bias):
    x = np.asarray(x)
    trellis = np.asarray(trellis).astype(np.uint16)
    suh = np.asarray(suh)
    svh = np.asarray(svh)
    bias = np.asarray(bias)

    nc = _build_program()
    in_maps = [
        _prep_core_inputs(x, trellis, suh, svh, bias, core) for core in range(NCORES)
    ]
    res = run_bass_kernel_spmd(nc, in_maps, core_ids=list(range(NCORES)))
    global LAST_RUN
    LAST_RUN = res
    out = np.concatenate([res.results[c]["out"] for c in range(NCORES)], axis=1)
    return out.astype(np.float16)


LAST_RUN = None


if __name__ == "__main__":
    import reference as ref
    import jax.numpy as jnp

    inputs = {k: np.asarray(v) for k, v in ref.setup_inputs().items()}
    expected = np.asarray(ref.reference(**{k: jnp.asarray(v) for k, v in inputs.items()}))
    got = kernel(**inputs)
    e = np.linalg.norm(got.astype(np.float32) - expected.astype(np.float32))
    n = np.linalg.norm(expected.astype(np.float32))
    print("Relative error:", e / n)
